# revision 37
# baseline (speedup 1.0000x reference)
"""Causal multi-head attention (B=2, L=2048, D=2048, H=32) on 8 trn2 NeuronCores.

Sharding: data-parallel over batch (2 groups of 4 cores) x tensor-parallel over
heads (8 heads per core). Each core computes, for its batch b and head range:
  qhT/khT = (W [dh,D]) @ x.T  (head dims on partitions, tokens on free axis)
  vh      = x @ W.T           (tokens on partitions: natural layout)
  S.T[k,q] = khT.T-block matmuls (contraction over head dim, K=64)
  P.T = exp(S.T) with causal masking (host-precomputed 128x128 triangle mask)
  o.T[d,q] accumulated over k-chunks; softmax denominator rides along as an
  appended ones-column of V on psum row 64; normalize via PE broadcast of the
  raw denominator + a parallel [64,512] approximate reciprocal (odd heads
  stage through SBUF + DMA into oT's upper partitions since matmul outputs
  must start at partition 0/32/64), then out.T = Wo_shard.T.T @ o.T
Host sums the 4 tensor-parallel partials per batch.

All matmul operands are bf16 (PSUM accumulation stays fp32): this enables the
PE's fast-weight-load path, halves DMA/SBUF/DVE traffic, and makes the
N=128 diagonal matmuls full-rate. Host pre-transposes and pre-interleaves
q/k/v/weights to partition-major layout so every DMA is wide and contiguous.
"""

import sys

sys.path.insert(0, "/opt/trn_rl_repo")

import numpy as np
import ml_dtypes

import concourse.bass as bass
import concourse.tile as tile
from concourse import bacc, mybir
from concourse.bass_utils import run_bass_kernel_spmd

BF16NP = ml_dtypes.bfloat16


def _ensure_ntff_hook():
    """The agent image's antenv package lacks axon_hooks, which makes
    run_bass_kernel_spmd(trace=True) crash on import. Provide the module and
    register the ctypes-based NTFF profiling hook (degrades silently)."""
    try:
        import types

        import antenv

        if "antenv.axon_hooks" not in sys.modules:
            m = types.ModuleType("antenv.axon_hooks")
            state = {"hook": None}
            m.set_axon_ntff_profile_hook = lambda h: state.__setitem__("hook", h)
            m.get_axon_ntff_profile_hook = lambda: state["hook"]
            sys.modules["antenv.axon_hooks"] = m
            antenv.axon_hooks = m
        from antenv.axon_hooks import (
            get_axon_ntff_profile_hook,
            set_axon_ntff_profile_hook,
        )

        if get_axon_ntff_profile_hook() is None:
            from trn_agent_boot.trn_boot import _ntff_profile_via_ctypes

            set_axon_ntff_profile_hook(
                _ntff_profile_via_ctypes("/opt/axon/libaxon_pjrt.so")
            )
    except Exception:
        pass


_ensure_ntff_hook()

F32 = mybir.dt.float32
F32R = mybir.dt.float32r
BF16 = mybir.dt.bfloat16

B, L, D, H = 2, 2048, 2048, 32
HD = 64          # head dim
N_CORES = 8
TP = 4           # tensor-parallel width (heads split 4 ways)
HPC = H // TP    # heads per core = 8
DH = HPC * HD    # per-core projected width = 512
SCALE = float(HD) ** -0.5

QB = 512         # query-block width for SDPA
XH = 1024        # token-half width for the projection streaming operand
KC = D // 128    # contraction chunks for the projections = 16
NQB = L // QB    # query blocks = 4
TC = L // 128    # 128-token chunks = 16
MCH = DH // 128  # head-pair chunks = 4
KPQ = QB // 128  # 128-k chunks per query block = 4
VW = HD + 1      # per-head vh width: [v(64), one]


def _emit(nc):
    xq = nc.dram_tensor("xq", [128, NQB * KC * QB], BF16, kind="ExternalInput")
    xk = nc.dram_tensor("xk", [128, NQB * KC * QB], BF16, kind="ExternalInput")
    xv = nc.dram_tensor("xv", [128, NQB * KC * QB], BF16, kind="ExternalInput")
    wq = nc.dram_tensor("wq", [128, KC * DH], BF16, kind="ExternalInput")
    wk = nc.dram_tensor("wk", [128, KC * DH], BF16, kind="ExternalInput")
    wv = nc.dram_tensor("wv", [128, KC * DH], BF16, kind="ExternalInput")
    wo = nc.dram_tensor("wo", [128, MCH * D], BF16, kind="ExternalInput")
    konst = nc.dram_tensor("konst", [128, 256], BF16, kind="ExternalInput")
    outT = nc.dram_tensor("outT", [D, L], BF16, kind="ExternalOutput")

    EXP = mybir.ActivationFunctionType.Exp

    with tile.TileContext(nc) as tc:
        from contextlib import ExitStack

        with ExitStack() as st:
            # ---- pools (everything coexists: projections are interleaved
            # into the SDPA stream as PE filler) ----
            wap = st.enter_context(tc.tile_pool(name="wall", bufs=1))
            xp = st.enter_context(tc.tile_pool(name="xp", bufs=2))
            constp = st.enter_context(tc.tile_pool(name="const", bufs=1))
            actp = st.enter_context(tc.tile_pool(name="acts", bufs=1))
            ppool = st.enter_context(tc.tile_pool(name="pp", bufs=12))
            dsbp = st.enter_context(tc.tile_pool(name="dsbp", bufs=2))
            recp = st.enter_context(tc.tile_pool(name="recp", bufs=2))
            stgp = st.enter_context(tc.tile_pool(name="stgp", bufs=3))
            osbp = st.enter_context(tc.tile_pool(name="osbp", bufs=3))
            spool = st.enter_context(tc.tile_pool(name="sps", bufs=2, space="PSUM"))
            opool = st.enter_context(tc.tile_pool(name="ops", bufs=2, space="PSUM"))
            fpool = st.enter_context(tc.tile_pool(name="fps", bufs=2, space="PSUM"))

            # ---- upfront DMAs: wq + first x quarter gate the first matmul,
            # so they go first; everything else overlaps the early compute ----
            wq_sb = wap.tile([128, KC, DH], BF16, tag="wq")
            wk_sb = wap.tile([128, KC, DH], BF16, tag="wk")
            wv_sb = wap.tile([128, KC, DH], BF16, tag="wv")
            wo_sb = wap.tile([128, MCH, D], BF16, tag="wo")

            def emit_wdma(wdram, wsb):
                wview = wdram[:].rearrange("p (kc m) -> p kc m", m=DH)
                nc.sync.dma_start(wsb[:], wview[:])

            xdrams = {"q": xq, "k": xk, "v": xv}
            wsbs = {"q": wq_sb, "k": wk_sb, "v": wv_sb}
            x_tiles = {}
            # quarter order: tokens 0:1024 of q/k/v first (enables SDPA
            # q-block groups 0-1), the rest are fillers inside groups 0-1
            QL = [("q", 0), ("q", 1), ("k", 0), ("k", 1), ("v", 0), ("v", 1),
                  ("q", 2), ("q", 3), ("k", 2), ("k", 3), ("v", 2), ("v", 3)]

            def emit_xdma(i):
                p, n2 = QL[i]
                x_sb = xp.tile([128, KC, QB], BF16, tag="x")
                xview = xdrams[p][
                    :, n2 * KC * QB : (n2 + 1) * KC * QB
                ].rearrange("p (kc t) -> p kc t", t=QB)
                if i <= 2:
                    # early quarters gate the pipeline start: split so matmul
                    # chains can begin on the first k-chunk groups
                    for g in range(0, KC, 4):
                        nc.sync.dma_start(x_sb[:, g : g + 4], xview[:, g : g + 4])
                else:
                    nc.sync.dma_start(x_sb[:], xview[:])
                x_tiles[i] = x_sb

            # interleave the wq / first-x-quarter splits so the very first
            # 16-matmul chain can start as soon as its first chunks land
            # the first weight/x quarter gate the very first matmul chain:
            # split them so the chain starts on the first 4 k-chunks while
            # the rest stream in (later quarters prefetch whole-tile)
            x0_sb = xp.tile([128, KC, QB], BF16, tag="x")
            x0view = xdrams["q"][:, 0 : KC * QB].rearrange(
                "p (kc t) -> p kc t", t=QB
            )
            wqview = wq[:].rearrange("p (kc m) -> p kc m", m=DH)
            for g in range(0, KC, 4):
                nc.sync.dma_start(wq_sb[:, g : g + 4], wqview[:, g : g + 4])
                nc.sync.dma_start(x0_sb[:, g : g + 4], x0view[:, g : g + 4])
            x_tiles[0] = x0_sb

            ksb = constp.tile([128, 256], BF16)
            nc.sync.dma_start(ksb[:], konst[:])
            tri_sb = ksb[:, 0:128]
            # the konst ones block doubles as the broadcast stationary operand
            ones_sb = ksb[:, 128:192]

            qhT = actp.tile([128, MCH, L], BF16)
            khT = actp.tile([128, MCH, L], BF16)
            # vh: per 128-token chunk, 8 heads x [v(64), one]
            vh = actp.tile([128, TC, HPC * VW], BF16)
            vh_ones = vh[:, :, :].rearrange("p t (h r) -> p (t h) r", r=VW)
            nc.vector.tensor_copy(
                vh_ones[:, :, HD : HD + 1],
                ksb[:, 128:256].rearrange("p (a b) -> p a b", b=1),
            )

            # ---- projection chunk: 16-matmul psum chain + cast-copy.
            # q/k: out[dim_chunk, tokens] = w_chunk.T @ xT (head dims on
            # partitions); v: natural layout, x chunk is the stationary op ----
            WEXTRA = {1: "k", 2: "v", 5: "o"}

            def proj_chunk(i, m):
                p, n2 = QL[i]
                if m == 0:
                    if i + 1 < len(QL):
                        emit_xdma(i + 1)  # one-quarter DMA lookahead
                    extra = WEXTRA.get(i)
                    if extra == "k":
                        emit_wdma(wk, wk_sb)
                    elif extra == "v":
                        emit_wdma(wv, wv_sb)
                    elif extra == "o":
                        woview = wo[:].rearrange("p (kc m) -> p kc m", m=D)
                        nc.sync.dma_start(wo_sb[:], woview[:])
                x_sb = x_tiles[i]
                w_sb = wsbs[p]
                ps = fpool.tile([128, QB], F32, tag="f", name="proj_ps")
                if p == "v":
                    for kc in range(KC):
                        nc.tensor.matmul(
                            ps[:],
                            x_sb[:, kc, m * 128 : (m + 1) * 128],
                            w_sb[:, kc, :],
                            start=(kc == 0),
                            stop=(kc == KC - 1),
                        )
                    tci = n2 * 4 + m
                    dstv = vh[:, tci, :].rearrange("p (h r) -> p h r", r=VW)
                    nc.vector.tensor_copy(
                        dstv[:, :, 0:HD],
                        ps[:].rearrange("p (h d) -> p h d", d=HD),
                    )
                else:
                    for kc in range(KC):
                        nc.tensor.matmul(
                            ps[:],
                            w_sb[:, kc, m * 128 : (m + 1) * 128],
                            x_sb[:, kc, :],
                            start=(kc == 0),
                            stop=(kc == KC - 1),
                        )
                    dst = qhT if p == "q" else khT
                    nc.vector.tensor_copy(dst[:, m, n2 * QB : (n2 + 1) * QB], ps[:])

            # ---- phase A: project tokens 0:1024 of q, k, v ----
            for i in range(6):
                for m in range(MCH):
                    proj_chunk(i, m)
            fillers = [(i, m) for i in range(6, 12) for m in range(MCH)]

            # ---- SDPA + output accumulation + fused projections ----
            otp = st.enter_context(tc.tile_pool(name="otp", bufs=1))
            oT = otp.tile([128, MCH, L], BF16)
            if True:
                # Software pipeline over (head, q-block) blocks so the PE
                # never stalls on the ACT exp latency: block B's o-matmuls
                # are interleaved with block B+1's score matmuls, and the
                # normalization (which waits on a DVE reciprocal) trails by
                # two blocks. Blocks are ordered qb-major so a query-block's
                # oT columns finish together. PE filler between steps keeps
                # the tensor engine p-state warm while Scalar's exp is the
                # per-iteration rate limiter: groups 0-1 interleave the
                # remaining projection quarters, groups 2-3 interleave the
                # output projection of q-blocks 0-1.
                blocks = [(h, qb) for qb in range(NQB) for h in range(HPC)]
                state = {}

                def kcnt_of(b):
                    return (b[1] + 1) * KPQ

                def emit_s_pair(b, j):
                    # Two 128-k chunks share one 2-bank psum tile and a single
                    # exp activation (halves Scalar op count + semaphore
                    # traffic). For diagonal pairs the gap between the two
                    # valid column spans may hold uninitialized psum; exp of
                    # it lands in p_sb columns no o-matmul ever reads.
                    h, qb = b
                    half, mch = 64 * (h % 2), h // 2
                    q0 = qb * QB
                    s_ps = spool.tile([128, 2 * QB], F32, tag="s", name="s_ps")
                    p_sb = ppool.tile([128, 2 * QB], BF16, tag="p", name="p_sb")
                    col0s = []
                    for t in (0, 1):
                        kc = 2 * j + t
                        dj = kc - qb * KPQ
                        col0 = 128 * dj if dj > 0 else 0
                        col0s.append(col0)
                        nc.tensor.matmul(
                            s_ps[:, t * QB + col0 : (t + 1) * QB],
                            khT[half : half + 64, mch, kc * 128 : (kc + 1) * 128],
                            qhT[half : half + 64, mch, q0 + col0 : q0 + QB],
                            start=True,
                            stop=True,
                        )
                    nc.scalar.activation(
                        p_sb[:, col0s[0] : 2 * QB], s_ps[:, col0s[0] : 2 * QB], EXP
                    )
                    for t in (0, 1):
                        kc = 2 * j + t
                        if kc >= qb * KPQ:
                            c = t * QB + col0s[t]
                            nc.vector.tensor_mul(
                                p_sb[:, c : c + 128],
                                p_sb[:, c : c + 128],
                                tri_sb[:],
                            )
                    state[b]["p"].append((p_sb, col0s))

                def emit_o_pair(b, j):
                    # One accumulation per head at psum base 0: 64 o-rows plus
                    # the denominator row from the ones-column of vh. Diagonal
                    # chunks only touch columns >= col0 (left of that is
                    # causally zero, so the exp/mask fill is skipped too).
                    h, qb = b
                    st_ = state[b]
                    if j == 0:
                        st_["o"] = opool.tile([128, QB], F32, tag="o", name="o_ps")
                    p_sb, col0s = st_["p"][j]
                    for t in (0, 1):
                        kc = 2 * j + t
                        col0 = col0s[t]
                        nc.tensor.matmul(
                            st_["o"][0:65, col0:QB],
                            vh[:, kc, h * VW : h * VW + VW],
                            p_sb[:, t * QB + col0 : (t + 1) * QB],
                            start=(kc == 0),
                            stop=(kc == kcnt_of(b) - 1),
                        )

                def emit_dcopy(b):
                    # raw denominator row -> SBUF (moving operand for the
                    # broadcast matmul)
                    st_ = state[b]
                    dsb = dsbp.tile([128, QB], BF16, tag="dsb", name="dsb")
                    nc.vector.tensor_copy(dsb[64:65, :], st_["o"][64:65, :])
                    st_["dsb"] = dsb

                def emit_norm(b):
                    h, qb = b
                    mch, q0 = h // 2, qb * QB
                    st_ = state.pop(b)
                    # bc shares the filler psum pool (it lives only until the
                    # reciprocal right after it)
                    bc_ps = fpool.tile([128, QB], F32, tag="f", name="bc_ps")
                    nc.tensor.matmul(
                        bc_ps[0:64, :],
                        ones_sb[64:65, :],
                        st_["dsb"][64:65, :],
                        start=True,
                        stop=True,
                    )
                    rec = recp.tile([128, QB], F32, tag="rec", name="rec")
                    nc.vector.reciprocal_approx_fast(
                        rec[0:64, :], bc_ps[0:64, :]
                    )
                    # At most one PSUM input per vector op: rec is in SBUF, o
                    # still in PSUM. Odd heads go to oT's upper partitions via
                    # a staging DMA (matmul outputs must start at 0/32/64).
                    if h % 2 == 0:
                        nc.vector.tensor_mul(
                            oT[0:64, mch, q0 : q0 + QB],
                            rec[0:64, :],
                            st_["o"][0:64, :],
                        )
                    else:
                        stg = stgp.tile([64, QB], BF16, tag="stg", name="stg")
                        nc.vector.tensor_mul(
                            stg[:], rec[0:64, :], st_["o"][0:64, :]
                        )
                        nc.sync.dma_start(oT[64:128, mch, q0 : q0 + QB], stg[:])

                def emit_oproj(n, m, pool=None, tag="f", on_scalar=False):
                    # outT[m, n-cols] = wo_chunk.T @ oT -- interleaved into
                    # the SDPA stream as PE filler once q-block n's oT
                    # columns are normalized.
                    pt = (pool or fpool).tile([128, QB], F32, tag=tag, name="oproj_ps")
                    for kc2 in range(MCH):
                        nc.tensor.matmul(
                            pt[:],
                            wo_sb[:, kc2, m * 128 : (m + 1) * 128],
                            oT[:, kc2, n * QB : (n + 1) * QB],
                            start=(kc2 == 0),
                            stop=(kc2 == MCH - 1),
                        )
                    osb = osbp.tile([128, QB], BF16, tag="ot")
                    if on_scalar:
                        # drain phase: Scalar is idle after the last exp, so
                        # psum evacuation there unblocks the pool recycle
                        # without queueing behind the DVE norm work
                        nc.scalar.activation(
                            osb[:], pt[:], mybir.ActivationFunctionType.Copy
                        )
                    else:
                        nc.vector.tensor_copy(osb[:], pt[:])
                    nc.sync.dma_start(
                        outT[m * 128 : (m + 1) * 128, n * QB : (n + 1) * QB],
                        osb[:],
                    )

                # per-step filler budget: groups 0-1 consume the 24
                # remaining projection chunks; group 2 absorbs out-proj of
                # q-blocks 0+1, group 3 takes q-block 2, the drain q-block 3.
                # Fillers are spread between the S/O pair emissions so the
                # shared psum pool has time to recycle behind the DVE copies.
                FILL = {0: 2, 1: 1, 2: 2, 3: 1, 4: 2, 5: 1, 6: 2, 7: 1}
                OPJ2 = {h: 4 for h in range(HPC)}
                OPJ3 = {2: 3, 3: 3, 4: 3, 5: 3, 6: 2, 7: 2}
                opj_q = [(n, m) for n in range(NQB) for m in range(D // 128)]
                seq = blocks + [None, None]
                for idx, b in enumerate(seq):
                    prev = seq[idx - 1] if idx >= 1 else None
                    prev2 = seq[idx - 2] if idx >= 2 else None
                    if b is not None:
                        state[b] = {"p": []}
                    ns = kcnt_of(b) // 2 if b is not None else 0
                    no = kcnt_of(prev) // 2 if prev is not None else 0

                    step_fill = []
                    if b is not None:
                        h, qb = b
                        if qb <= 1:
                            for _ in range(FILL[h]):
                                if fillers:
                                    ii, mm = fillers.pop(0)
                                    step_fill.append(
                                        lambda ii=ii, mm=mm: proj_chunk(ii, mm)
                                    )
                        else:
                            cnt = OPJ2[h] if qb == 2 else OPJ3.get(h, 0)
                            for _ in range(cnt):
                                if opj_q:
                                    n, m = opj_q.pop(0)
                                    step_fill.append(
                                        lambda n=n, m=m: emit_oproj(n, m)
                                    )

                    # group two S-pairs then two O-pairs: S matmuls run the
                    # PE in 64-row tiling mode while O/proj run 128-row mode,
                    # and every mode switch drains the array -- pairing
                    # same-mode matmuls halves the switch count
                    for i2 in range(0, max(ns, no), 2):
                        for i in (i2, i2 + 1):
                            if i < ns:
                                emit_s_pair(b, i)
                        for i in (i2, i2 + 1):
                            if i < no:
                                emit_o_pair(prev, i)
                        if step_fill:
                            step_fill.pop(0)()
                    for f in step_fill:
                        f()
                    if prev is not None:
                        emit_dcopy(prev)
                    if prev2 is not None:
                        emit_norm(prev2)
                # drain: the last q-block's out-proj rotates across all three
                # psum pools so the PE never waits on a single pool's copies
                drain_pools = [fpool, spool, opool]
                drain_tags = ["f", "s", "o"]
                for di, (n, m) in enumerate(opj_q):
                    emit_oproj(
                        n, m, drain_pools[di % 3], drain_tags[di % 3],
                        on_scalar=(di % 2 == 1),
                    )
    return nc


def build():
    nc = bacc.Bacc("TRN2", target_bir_lowering=False, debug=False)
    _emit(nc)
    nc.compile()
    return nc


_NC_CACHE = {}


def _get_nc():
    if "nc" not in _NC_CACHE:
        _NC_CACHE["nc"] = build()
    return _NC_CACHE["nc"]


def _interleave_x(xT):
    """[D, L] fp32 -> [128, NQB*KC*QB] bf16 laid out (p, quarter, kc, t)."""
    r = xT.reshape(KC, 128, NQB, QB).transpose(1, 2, 0, 3)
    return np.ascontiguousarray(r.reshape(128, NQB * KC * QB).astype(BF16NP))


def _interleave_w(w, m):
    """[D_contract, m] fp32 -> [128, (D_contract/128)*m] bf16 (p, kc, m)."""
    kc = w.shape[0] // 128
    r = w.reshape(kc, 128, m).transpose(1, 0, 2)
    return np.ascontiguousarray(r.reshape(128, kc * m).astype(BF16NP))


def make_in_maps(q, k, v, Wq, Wk, Wv, Wo):
    konst_m = np.ones((128, 256), dtype=np.float32).astype(BF16NP)
    konst_m[:, 0:128] = np.triu(np.ones((128, 128), dtype=np.float32)).astype(
        BF16NP
    )
    qT = [_interleave_x(q[b].T) for b in range(B)]
    kT = [_interleave_x(k[b].T) for b in range(B)]
    vT = [_interleave_x(v[b].T) for b in range(B)]
    wq_s, wk_s, wv_s, wo_s = [], [], [], []
    for tp in range(TP):
        rows = slice(tp * DH, (tp + 1) * DH)
        wq_s.append(_interleave_w(np.ascontiguousarray(Wq[rows].T) * SCALE, DH))
        wk_s.append(_interleave_w(np.ascontiguousarray(Wk[rows].T), DH))
        wv_s.append(_interleave_w(np.ascontiguousarray(Wv[rows].T), DH))
        wo_s.append(_interleave_w(np.ascontiguousarray(Wo[:, rows].T), D))
    in_maps = []
    for c in range(N_CORES):
        b, tp = c // TP, c % TP
        in_maps.append(
            {
                "xq": qT[b],
                "xk": kT[b],
                "xv": vT[b],
                "wq": wq_s[tp],
                "wk": wk_s[tp],
                "wv": wv_s[tp],
                "wo": wo_s[tp],
                "konst": konst_m,
            }
        )
    return in_maps


def kernel(q, k, v, Wq, Wk, Wv, Wo, mask=None, trace=False):
    q = np.asarray(q, dtype=np.float32)
    k = np.asarray(k, dtype=np.float32)
    v = np.asarray(v, dtype=np.float32)
    nc = _get_nc()
    in_maps = make_in_maps(
        q, k, v,
        np.asarray(Wq, np.float32), np.asarray(Wk, np.float32),
        np.asarray(Wv, np.float32), np.asarray(Wo, np.float32),
    )
    res = run_bass_kernel_spmd(
        nc, in_maps, core_ids=list(range(N_CORES)), trace=trace
    )
    out = np.zeros((B, L, D), dtype=np.float32)
    for c in range(N_CORES):
        out[c // TP] += res.results[c]["outT"].T.astype(np.float32)
    if trace:
        return out, res
    return out
